# revision 30
# baseline (speedup 1.0000x reference)
"""Trainium2 Bass kernel for nn_ChunkedConvblock (chunked conv + local attention + LN + pool).

Reference computation per chunk of L=64 timesteps (D=512):
  ct = relu(conv1d(x^T, k=2, dilation=2, pad=1) + cb)^T     # [L, D]
  scores = (x @ ct^T) / sqrt(D); p = softmax(scores, -1)
  h = LN(p @ ct + ct) * g + b ; out = mean_t(h)             # [D]

Distribution: data-parallel over batch B=32 across 8 cores (4 rows/core).
Each core processes 256 chunks as 32 "supertiles" of 8 chunks (512 steps).

Layout/precision strategy:
  - x is shipped as fp16 and loaded pre-transposed via HWDGE DMA-transpose
    (d on partitions), so no PE transposes or PSUM evacuations are needed
    for q^T / the conv inputs.
  - conv / scores / attention matmuls run in fp16 (fp32 PSUM accumulate);
    softmax statistics, layernorm, and pooling run in fp32/f32r.
  - layernorm gain/bias commute with the time-mean, so they are applied
    once per chunk after pooling instead of per element.
  - rsqrt for LN = exp(-0.5*ln(var+eps)) keeps every ACT func in one
    activation-table set (no table reloads).
"""

import sys

if "/opt/trn_rl_repo" not in sys.path:
    sys.path.insert(0, "/opt/trn_rl_repo")

import numpy as np

import concourse.bass as bass
from concourse.bacc import Bacc
from concourse import mybir
from concourse.tile import TileContext
from concourse.bass_utils import run_bass_kernel_spmd

F32 = mybir.dt.float32
F32R = mybir.dt.float32r
F16 = mybir.dt.float16
AF = mybir.ActivationFunctionType
ALU = mybir.AluOpType

D = 512
L = 64
SUP = 512          # timesteps per supertile
CH = SUP // L      # 8 chunks per supertile


def build_program(B_loc: int, T: int, reps: int = 1, debug: bool = False):
    """Build the per-core Bass program. x local shape [B_loc, T, D] (fp16)."""
    n_sup_row = T // SUP
    n_chunks = B_loc * T // L

    nc = Bacc()
    x = nc.declare_dram_parameter("x", [B_loc, T, D], F16, isOutput=False)
    wt = nc.declare_dram_parameter("wt", [2 * D, D], F16, isOutput=False)
    cb = nc.declare_dram_parameter("cb", [D], F32, isOutput=False)
    g = nc.declare_dram_parameter("g", [D], F32, isOutput=False)
    bb = nc.declare_dram_parameter("bb", [D], F32, isOutput=False)
    pw = nc.declare_dram_parameter("pw", [128, 32], F32, isOutput=False)
    ident = nc.declare_dram_parameter("ident", [128, 128], F16, isOutput=False)
    out = nc.declare_dram_parameter("out", [n_chunks, D], F32, isOutput=True)
    dbg = {}
    if debug:
        dbg["xq"] = nc.declare_dram_parameter("d_xq", [128, 4, SUP], F16, isOutput=True)
        dbg["xts"] = nc.declare_dram_parameter("d_xts", [128, 4, 2, SUP], F16, isOutput=True)
        dbg["ctT"] = nc.declare_dram_parameter("d_ctT", [128, 4, SUP], F16, isOutput=True)
        dbg["ctn"] = nc.declare_dram_parameter("d_ctn", [128, 4, D], F16, isOutput=True)
        dbg["p"] = nc.declare_dram_parameter("d_p", [4, 128, 128], F16, isOutput=True)
        dbg["pT"] = nc.declare_dram_parameter("d_pT", [4, 128, 128], F16, isOutput=True)
        dbg["den"] = nc.declare_dram_parameter("d_den", [128, 4], F32, isOutput=True)
        dbg["mv"] = nc.declare_dram_parameter("d_mv", [128, 4, 2], F32, isOutput=True)
        dbg["rstd"] = nc.declare_dram_parameter("d_rstd", [128, 4], F32, isOutput=True)
        dbg["a"] = nc.declare_dram_parameter("d_a", [4, 128, D], F32, isOutput=True)

    inv_sqrt_d = float(1.0 / np.sqrt(D))
    from contextlib import nullcontext

    with TileContext(nc) as tc, \
         tc.tile_pool(name="singles", bufs=1) as singles, \
         tc.tile_pool(name="xq", bufs=3) as xq_pool, \
         tc.tile_pool(name="xT", bufs=3) as xT_pool, \
         tc.tile_pool(name="ctT", bufs=3) as ctT_pool, \
         tc.tile_pool(name="ctn", bufs=3) as ctn_pool, \
         tc.tile_pool(name="soft", bufs=6) as soft_pool, \
         tc.tile_pool(name="acc", bufs=4) as acc_pool, \
         tc.tile_pool(name="stat", bufs=8) as stat_pool, \
         tc.tile_pool(name="po", bufs=2) as po_pool, \
         tc.tile_pool(name="ps", bufs=6, space="PSUM") as ps_pool, \
         tc.tile_pool(name="psp", bufs=2, space="PSUM") as psp_pool:

        # ---- one-time constants ----
        wt_sb = singles.tile([128, 8, D], F16)
        nc.sync.dma_start(out=wt_sb, in_=wt.rearrange("(p k) o -> p k o", p=128))
        ident_sb = singles.tile([128, 128], F16)
        nc.sync.dma_start(out=ident_sb, in_=ident[:, :])
        cb_sb = singles.tile([128, 4], F32)
        nc.sync.dma_start(out=cb_sb, in_=cb.rearrange("(j p) -> p j", p=128))
        pw_sb = singles.tile([128, 32], F32)
        nc.sync.dma_start(out=pw_sb, in_=pw[:, :])
        g_ap = g[:]
        b_ap = bb[:]
        eps_sb = singles.tile([128, 1], F32)
        nc.vector.memset(eps_sb, 1e-5)
        g8 = singles.tile([8, D], F32)
        nc.sync.dma_start(
            out=g8,
            in_=bass.AP(tensor=g_ap.tensor, offset=g_ap.offset,
                        ap=[[0, 8]] + list(g_ap.ap)),
        )
        b8 = singles.tile([8, D], F32)
        nc.sync.dma_start(
            out=b8,
            in_=bass.AP(tensor=b_ap.tensor, offset=b_ap.offset,
                        ap=[[0, 8]] + list(b_ap.ap)),
        )
        # persistent softmax tiles: off-diagonal (cross-chunk) blocks are
        # zeroed once; the per-supertile exps only rewrite the diagonals
        p_pers = []
        for i in range(4):
            p_sb = singles.tile([128, 128], F16, tag=f"p{i}")
            nc.vector.memset(p_sb[0:64, 64:128], 0.0)
            nc.vector.memset(p_sb[64:128, 0:64], 0.0)
            p_pers.append(p_sb)

        with (tc.For_i(0, reps, 1) if reps > 1 else nullcontext()):
            pending_tail = None
            for row in range(B_loc):
                for s in range(n_sup_row):
                    t0 = s * SUP
                    # ---- load x^T directly via DMA-transpose: [128(d), db, t] ----
                    xq = xq_pool.tile([128, 4, SUP], F16, tag="xq")
                    for db in range(4):
                        nc.sync.dma_start(
                            out=xq[:, db, :],
                            in_=x[row, t0:t0 + SUP, db * 128:(db + 1) * 128],
                            transpose=True,
                        )
                    xq_c = xq.rearrange("p n (c w) -> p n c w", w=64)

                    # ---- conv-tap shifted copies (zero at chunk boundaries) ----
                    # v=0: col t holds x[t-1]; v=1: col t holds x[t+1]
                    xTs = xT_pool.tile([128, 4, 2, SUP], F16, tag="xTs")
                    xTs_c = xTs.rearrange("p n v (c w) -> p n v c w", w=64)
                    nc.vector.memset(xTs_c[:, :, 0, :, 0], 0.0)
                    nc.vector.memset(xTs_c[:, :, 1, :, 63], 0.0)
                    for db in range(4):
                        nc.vector.tensor_copy(
                            out=xTs_c[:, db, 0, :, 1:64], in_=xq_c[:, db, :, 0:63]
                        )
                        nc.vector.tensor_copy(
                            out=xTs_c[:, db, 1, :, 0:63], in_=xq_c[:, db, :, 1:64]
                        )

                    if debug and row == 0 and s == 0:
                        nc.sync.dma_start(out=dbg["xq"][:, :, :], in_=xq)
                        nc.sync.dma_start(out=dbg["xts"][:, :, :, :], in_=xTs)

                    # ---- conv: ctT[o_blk] = relu(W*x + cb), [o(d) on partitions] ----
                    ctT = ctT_pool.tile([128, 4, SUP], F16, tag="ctT")
                    for ob in range(4):
                        psc = ps_pool.tile([128, SUP], F32, tag="ps")
                        k = 0
                        for tap in range(2):
                            for ib in range(4):
                                nc.tensor.matmul(
                                    psc,
                                    wt_sb[:, tap * 4 + ib, ob * 128:(ob + 1) * 128],
                                    xTs[:, ib, tap, :],
                                    start=(k == 0),
                                    stop=(k == 7),
                                )
                                k += 1
                        nc.scalar.activation(
                            out=ctT[:, ob, :], in_=psc, func=AF.Relu,
                            bias=cb_sb[:, ob:ob + 1], scale=1.0,
                        )

                    if pending_tail is not None:
                        pending_tail()
                        pending_tail = None

                    # ---- transpose ctT -> ct natural [t on partitions] ----
                    ctn = ctn_pool.tile([128, 4, D], F16, tag="ctn")
                    for tb in range(4):
                        ps2 = ps_pool.tile([128, D], F16, tag="ps")
                        for db in range(4):
                            nc.tensor.transpose(
                                ps2[:, db * 128:(db + 1) * 128],
                                ctT[:, db, tb * 128:(tb + 1) * 128],
                                ident_sb,
                            )
                        nc.vector.tensor_copy(out=ctn[:, tb, :], in_=ps2)

                    if debug and row == 0 and s == 0:
                        nc.sync.dma_start(out=dbg["ctT"][:, :, :], in_=ctT)
                        nc.sync.dma_start(out=dbg["ctn"][:, :, :], in_=ctn)

                    # ---- per chunk-pair: scores, softmax, attn, LN, pool ----
                    # software-pipelined: pT lags scores by 1 pair, attn by 2,
                    # so PE never head-of-line blocks on the ACT exp/copy hops
                    den_all = stat_pool.tile([128, 4], F32, tag="den")
                    rec_all = stat_pool.tile([128, 4], F32, tag="rec")
                    mv_all = stat_pool.tile([128, 4, 2], F32, tag="mv")
                    rstd_all = stat_pool.tile([128, 4], F32, tag="rstd")
                    lnv_all = stat_pool.tile([128, 4], F32, tag="lnv")
                    pT_tiles = [None] * 4
                    pss_tiles = [None] * 4
                    a_tiles = [None] * 4

                    def scores_exp(pr):
                        pss = ps_pool.tile([128, 128], F32, tag="ps")
                        for db in range(4):
                            nc.tensor.matmul(
                                pss,
                                xq[:, db, pr * 128:(pr + 1) * 128],
                                ctT[:, db, pr * 128:(pr + 1) * 128],
                                start=(db == 0), stop=(db == 3),
                            )
                        p_sb = p_pers[pr]
                        nc.scalar.activation(
                            out=p_sb[0:64, 0:64], in_=pss[0:64, 0:64],
                            func=AF.Exp, scale=inv_sqrt_d,
                            accum_out=den_all[0:64, pr:pr + 1],
                        )
                        nc.scalar.activation(
                            out=p_sb[64:128, 64:128], in_=pss[64:128, 64:128],
                            func=AF.Exp, scale=inv_sqrt_d,
                            accum_out=den_all[64:128, pr:pr + 1],
                        )
                        nc.vector.reciprocal(
                            rec_all[:, pr:pr + 1], den_all[:, pr:pr + 1]
                        )
                        pss_tiles[pr] = pss

                    def p_transpose(pr):
                        ps3 = ps_pool.tile([128, 128], F16, tag="ps")
                        nc.tensor.transpose(ps3, p_pers[pr], ident_sb)
                        pT_sb = soft_pool.tile([128, 128], F16, tag="pT")
                        nc.scalar.copy(out=pT_sb, in_=ps3)
                        pT_tiles[pr] = pT_sb

                    def attn_stats(pr):
                        psa = ps_pool.tile([128, D], F32, tag="ps")
                        nc.tensor.matmul(psa, pT_tiles[pr], ctn[:, pr, :])
                        a_sb = acc_pool.tile([128, D], F32, tag="a")
                        nc.vector.scalar_tensor_tensor(
                            out=a_sb, in0=psa, scalar=rec_all[:, pr:pr + 1],
                            in1=ctn[:, pr, :], op0=ALU.mult, op1=ALU.add,
                        )
                        st = stat_pool.tile([128, 6], F32, tag="st")
                        nc.vector.bn_stats(st, a_sb)
                        nc.vector.bn_aggr(mv_all[:, pr, :], st)
                        a_tiles[pr] = a_sb
                        if debug and row == 0 and s == 0:
                            nc.sync.dma_start(out=dbg["p"][pr, :, :], in_=p_pers[pr])
                            nc.sync.dma_start(out=dbg["pT"][pr, :, :], in_=pT_tiles[pr])

                    PIPE = True
                    if PIPE:
                        for step in range(6):
                            if step < 4:
                                scores_exp(step)
                            if 1 <= step <= 4:
                                p_transpose(step - 1)
                            if step >= 2:
                                attn_stats(step - 2)
                    else:
                        for pr in range(4):
                            scores_exp(pr)
                            p_transpose(pr)
                            attn_stats(pr)

                    def make_tail(mv_all, lnv_all, rstd_all, a_tiles, chunk0):
                        def tail():
                            nc.scalar.activation(
                                out=lnv_all, in_=mv_all[:, :, 1], func=AF.Sqrt,
                                bias=eps_sb, scale=1.0,
                            )
                            nc.vector.reciprocal(rstd_all, lnv_all)
                            psp = psp_pool.tile([8, D], F32, tag="psp")
                            if debug and chunk0 == 0:
                                nc.sync.dma_start(out=dbg["rstd"][:, :], in_=rstd_all)
                            for pr in range(4):
                                a_sb = a_tiles[pr]
                                nc.vector.tensor_scalar(
                                    out=a_sb, in0=a_sb, scalar1=mv_all[:, pr, 0:1],
                                    scalar2=rstd_all[:, pr:pr + 1],
                                    op0=ALU.subtract, op1=ALU.mult,
                                )
                                nc.tensor.matmul(
                                    psp, pw_sb[:, pr * 8:(pr + 1) * 8], a_sb,
                                    start=(pr == 0), stop=(pr == 3),
                                )
                                if debug and chunk0 == 0:
                                    nc.sync.dma_start(
                                        out=dbg["a"][pr, :, :], in_=a_sb.bitcast(F32)
                                    )
                            out_sb = po_pool.tile([8, D], F32, tag="po")
                            nc.vector.tensor_copy(out=out_sb, in_=psp)
                            nc.vector.tensor_mul(out_sb, out_sb, g8)
                            nc.vector.tensor_add(out_sb, out_sb, b8)
                            nc.sync.dma_start(
                                out=out[chunk0:chunk0 + CH, :], in_=out_sb
                            )
                        return tail

                    if debug and row == 0 and s == 0:
                        nc.sync.dma_start(out=dbg["den"][:, :], in_=den_all)
                        nc.sync.dma_start(out=dbg["mv"][:, :, :], in_=mv_all)
                    chunk0 = row * (T // L) + s * CH
                    pending_tail = make_tail(
                        mv_all, lnv_all, rstd_all, list(a_tiles), chunk0
                    )
            if pending_tail is not None:
                pending_tail()
                pending_tail = None

    nc.finalize()
    return nc


def host_inputs(x, conv_w, conv_b, ln_g, ln_b, n_cores):
    """Shard + transform inputs for the device program."""
    B = x.shape[0]
    b_loc = B // n_cores
    wt = np.transpose(np.asarray(conv_w, dtype=np.float32), (2, 1, 0)).reshape(
        2 * D, D
    ).astype(np.float16)
    # device loads wt as [p, k, o] with row index p*8+k: row p*8+k must hold
    # original wt row k*128+p (k = tap*4 + i_block)
    idx = (np.arange(1024) % 8) * 128 + (np.arange(1024) // 8)
    wt = np.ascontiguousarray(wt[idx])
    pw = np.zeros((128, 32), dtype=np.float32)
    for tp in range(128):
        for p in range(4):
            pw[tp, p * 8 + 2 * p + tp // 64] = 1.0 / 64
    ident = np.eye(128, dtype=np.float16)
    x16 = np.asarray(x, dtype=np.float16)
    common = {
        "wt": wt,
        "cb": np.ascontiguousarray(np.asarray(conv_b, dtype=np.float32)),
        "g": np.ascontiguousarray(np.asarray(ln_g, dtype=np.float32)),
        "bb": np.ascontiguousarray(np.asarray(ln_b, dtype=np.float32)),
        "pw": pw,
        "ident": ident,
    }
    in_maps = []
    for c in range(n_cores):
        m = dict(common)
        m["x"] = np.ascontiguousarray(x16[c * b_loc:(c + 1) * b_loc])
        in_maps.append(m)
    return in_maps


def kernel(x, conv_w, conv_b, ln_g, ln_b, n_stages):
    x = np.asarray(x)
    B, T, d = x.shape
    assert d == D and int(n_stages) * L == T, (x.shape, n_stages)
    n_cores = 8
    nc = build_program(B // n_cores, T, reps=1)
    in_maps = host_inputs(x, conv_w, conv_b, ln_g, ln_b, n_cores)
    res = run_bass_kernel_spmd(nc, in_maps, list(range(n_cores)))
    outs = [res.results[c]["out"] for c in range(n_cores)]
    full = np.concatenate(outs, axis=0)  # [B*n_stages, D]
    return full.reshape(B, int(n_stages), D).astype(np.float32)


if __name__ == "__main__":
    rng = np.random.default_rng(0)
    x = rng.standard_normal((32, 4096, D), dtype=np.float32)
    conv_w = (rng.standard_normal((D, D, 2)) / np.sqrt(2 * D)).astype(np.float32)
    conv_b = (rng.standard_normal(D) * 0.02).astype(np.float32)
    out = kernel(x, conv_w, conv_b, np.ones(D, np.float32), np.zeros(D, np.float32), 64)
    print(out.shape, out.dtype)


# revision 31
# speedup vs baseline: 1.4185x; 1.4185x over previous
"""Trainium2 Bass kernel for nn_ChunkedConvblock (chunked conv + local attention + LN + pool).

Reference computation per chunk of L=64 timesteps (D=512):
  ct = relu(conv1d(x^T, k=2, dilation=2, pad=1) + cb)^T     # [L, D]
  scores = (x @ ct^T) / sqrt(D); p = softmax(scores, -1)
  h = LN(p @ ct + ct) * g + b ; out = mean_t(h)             # [D]

Distribution: data-parallel over batch B=32 across 8 cores (4 rows/core).
Each core processes 256 chunks as 32 "supertiles" of 8 chunks (512 steps).

Layout/precision strategy:
  - x is shipped as fp16 and loaded pre-transposed via HWDGE DMA-transpose
    (d on partitions), so no PE transposes or PSUM evacuations are needed
    for q^T / the conv inputs.
  - conv / scores / attention matmuls run in fp16 (fp32 PSUM accumulate);
    softmax statistics, layernorm, and pooling run in fp32/f32r.
  - layernorm gain/bias commute with the time-mean, so they are applied
    once per chunk after pooling instead of per element.
  - rsqrt for LN = exp(-0.5*ln(var+eps)) keeps every ACT func in one
    activation-table set (no table reloads).
"""

import sys

if "/opt/trn_rl_repo" not in sys.path:
    sys.path.insert(0, "/opt/trn_rl_repo")

import numpy as np

import concourse.bass as bass
from concourse.bacc import Bacc
from concourse import mybir
from concourse.tile import TileContext
from concourse.bass_utils import run_bass_kernel_spmd

F32 = mybir.dt.float32
F32R = mybir.dt.float32r
F16 = mybir.dt.float16
AF = mybir.ActivationFunctionType
ALU = mybir.AluOpType

D = 512
L = 64
SUP = 512          # timesteps per supertile
CH = SUP // L      # 8 chunks per supertile


def build_program(B_loc: int, T: int, reps: int = 1, debug: bool = False):
    """Build the per-core Bass program. x local shape [B_loc, T, D] (fp16)."""
    n_sup_row = T // SUP
    n_chunks = B_loc * T // L

    nc = Bacc()
    x = nc.declare_dram_parameter("x", [B_loc, T, D], F16, isOutput=False)
    wt = nc.declare_dram_parameter("wt", [2 * D, D], F16, isOutput=False)
    cb = nc.declare_dram_parameter("cb", [D], F32, isOutput=False)
    g = nc.declare_dram_parameter("g", [D], F32, isOutput=False)
    bb = nc.declare_dram_parameter("bb", [D], F32, isOutput=False)
    pw = nc.declare_dram_parameter("pw", [128, 32], F16, isOutput=False)
    ident = nc.declare_dram_parameter("ident", [128, 128], F16, isOutput=False)
    out = nc.declare_dram_parameter("out", [n_chunks, D], F32, isOutput=True)
    dbg = {}
    if debug:
        dbg["xq"] = nc.declare_dram_parameter("d_xq", [128, 4, SUP], F16, isOutput=True)
        dbg["xts"] = nc.declare_dram_parameter("d_xts", [128, 4, 2, SUP], F16, isOutput=True)
        dbg["ctT"] = nc.declare_dram_parameter("d_ctT", [128, 4, SUP], F16, isOutput=True)
        dbg["ctn"] = nc.declare_dram_parameter("d_ctn", [128, 4, D], F16, isOutput=True)
        dbg["p"] = nc.declare_dram_parameter("d_p", [4, 128, 128], F16, isOutput=True)
        dbg["pT"] = nc.declare_dram_parameter("d_pT", [4, 128, 128], F16, isOutput=True)
        dbg["den"] = nc.declare_dram_parameter("d_den", [128, 4], F32, isOutput=True)
        dbg["mv"] = nc.declare_dram_parameter("d_mv", [128, 4, 2], F32, isOutput=True)
        dbg["rstd"] = nc.declare_dram_parameter("d_rstd", [128, 4], F32, isOutput=True)
        dbg["a"] = nc.declare_dram_parameter("d_a", [4, 128, D], F32, isOutput=True)

    inv_sqrt_d = float(1.0 / np.sqrt(D))
    from contextlib import nullcontext

    with TileContext(nc) as tc, \
         tc.tile_pool(name="singles", bufs=1) as singles, \
         tc.tile_pool(name="xq", bufs=3) as xq_pool, \
         tc.tile_pool(name="xT", bufs=3) as xT_pool, \
         tc.tile_pool(name="ctT", bufs=3) as ctT_pool, \
         tc.tile_pool(name="ctn", bufs=3) as ctn_pool, \
         tc.tile_pool(name="soft", bufs=6) as soft_pool, \
         tc.tile_pool(name="acc", bufs=4) as acc_pool, \
         tc.tile_pool(name="stat", bufs=8) as stat_pool, \
         tc.tile_pool(name="po", bufs=2) as po_pool, \
         tc.tile_pool(name="ps", bufs=6, space="PSUM") as ps_pool, \
         tc.tile_pool(name="psp", bufs=2, space="PSUM") as psp_pool:

        # ---- one-time constants ----
        wt_sb = singles.tile([128, 8, D], F16)
        nc.sync.dma_start(out=wt_sb, in_=wt.rearrange("(p k) o -> p k o", p=128))
        ident_sb = singles.tile([128, 128], F16)
        nc.sync.dma_start(out=ident_sb, in_=ident[:, :])
        cb_sb = singles.tile([128, 4], F32)
        nc.sync.dma_start(out=cb_sb, in_=cb.rearrange("(j p) -> p j", p=128))
        pw_sb = singles.tile([128, 32], F16)
        nc.sync.dma_start(out=pw_sb, in_=pw[:, :])
        g_ap = g[:]
        b_ap = bb[:]
        eps_sb = singles.tile([128, 1], F32)
        nc.vector.memset(eps_sb, 1e-5)
        g8 = singles.tile([8, D], F32)
        nc.sync.dma_start(
            out=g8,
            in_=bass.AP(tensor=g_ap.tensor, offset=g_ap.offset,
                        ap=[[0, 8]] + list(g_ap.ap)),
        )
        b8 = singles.tile([8, D], F32)
        nc.sync.dma_start(
            out=b8,
            in_=bass.AP(tensor=b_ap.tensor, offset=b_ap.offset,
                        ap=[[0, 8]] + list(b_ap.ap)),
        )
        # persistent softmax tiles: off-diagonal (cross-chunk) blocks are
        # zeroed once; the per-supertile exps only rewrite the diagonals
        p_pers = []
        for i in range(4):
            p_sb = singles.tile([128, 128], F16, tag=f"p{i}")
            nc.vector.memset(p_sb[0:64, 64:128], 0.0)
            nc.vector.memset(p_sb[64:128, 0:64], 0.0)
            p_pers.append(p_sb)

        with (tc.For_i(0, reps, 1) if reps > 1 else nullcontext()):
            pending_tail = None
            for row in range(B_loc):
                for s in range(n_sup_row):
                    t0 = s * SUP
                    # ---- load x^T directly via DMA-transpose: [128(d), db, t] ----
                    xq = xq_pool.tile([128, 4, SUP], F16, tag="xq")
                    for db in range(4):
                        nc.sync.dma_start(
                            out=xq[:, db, :],
                            in_=x[row, t0:t0 + SUP, db * 128:(db + 1) * 128],
                            transpose=True,
                        )
                    xq_c = xq.rearrange("p n (c w) -> p n c w", w=64)

                    # ---- conv-tap shifted copies (zero at chunk boundaries) ----
                    # v=0: col t holds x[t-1]; v=1: col t holds x[t+1]
                    xTs = xT_pool.tile([128, 4, 2, SUP], F16, tag="xTs")
                    xTs_c = xTs.rearrange("p n v (c w) -> p n v c w", w=64)
                    nc.vector.memset(xTs_c[:, :, 0, :, 0], 0.0)
                    nc.vector.memset(xTs_c[:, :, 1, :, 63], 0.0)
                    for db in range(4):
                        nc.scalar.copy(
                            out=xTs_c[:, db, 0, :, 1:64], in_=xq_c[:, db, :, 0:63]
                        )
                        nc.vector.tensor_copy(
                            out=xTs_c[:, db, 1, :, 0:63], in_=xq_c[:, db, :, 1:64]
                        )

                    if debug and row == 0 and s == 0:
                        nc.sync.dma_start(out=dbg["xq"][:, :, :], in_=xq)
                        nc.sync.dma_start(out=dbg["xts"][:, :, :, :], in_=xTs)

                    # ---- conv: ctT[o_blk] = relu(W*x + cb), [o(d) on partitions] ----
                    ctT = ctT_pool.tile([128, 4, SUP], F16, tag="ctT")
                    for ob in range(4):
                        psc = ps_pool.tile([128, SUP], F32, tag="ps")
                        k = 0
                        for tap in range(2):
                            for ib in range(4):
                                nc.tensor.matmul(
                                    psc,
                                    wt_sb[:, tap * 4 + ib, ob * 128:(ob + 1) * 128],
                                    xTs[:, ib, tap, :],
                                    start=(k == 0),
                                    stop=(k == 7),
                                )
                                k += 1
                        nc.scalar.activation(
                            out=ctT[:, ob, :], in_=psc, func=AF.Relu,
                            bias=cb_sb[:, ob:ob + 1], scale=1.0,
                        )

                    if pending_tail is not None:
                        pending_tail()
                        pending_tail = None

                    # ---- transpose ctT -> ct natural [t on partitions] ----
                    ctn = ctn_pool.tile([128, 4, D], F16, tag="ctn")
                    for tb in range(4):
                        ps2 = ps_pool.tile([128, D], F16, tag="ps")
                        for db in range(4):
                            nc.tensor.transpose(
                                ps2[:, db * 128:(db + 1) * 128],
                                ctT[:, db, tb * 128:(tb + 1) * 128],
                                ident_sb,
                            )
                        nc.vector.tensor_copy(out=ctn[:, tb, :], in_=ps2)

                    if debug and row == 0 and s == 0:
                        nc.sync.dma_start(out=dbg["ctT"][:, :, :], in_=ctT)
                        nc.sync.dma_start(out=dbg["ctn"][:, :, :], in_=ctn)

                    # ---- per chunk-pair: scores, softmax, attn, LN, pool ----
                    # software-pipelined: pT lags scores by 1 pair, attn by 2,
                    # so PE never head-of-line blocks on the ACT exp/copy hops
                    den_all = stat_pool.tile([128, 4], F32, tag="den")
                    rec_all = stat_pool.tile([128, 4], F32, tag="rec")
                    mv_all = stat_pool.tile([128, 4, 2], F32, tag="mv")
                    rstd_all = stat_pool.tile([128, 4], F32, tag="rstd")
                    lnv_all = stat_pool.tile([128, 4], F32, tag="lnv")
                    pT_tiles = [None] * 4
                    pss_tiles = [None] * 4
                    a_tiles = [None] * 4

                    def scores_exp(pr):
                        pss = ps_pool.tile([128, 128], F32, tag="ps")
                        for db in range(4):
                            nc.tensor.matmul(
                                pss,
                                xq[:, db, pr * 128:(pr + 1) * 128],
                                ctT[:, db, pr * 128:(pr + 1) * 128],
                                start=(db == 0), stop=(db == 3),
                            )
                        p_sb = p_pers[pr]
                        nc.scalar.activation(
                            out=p_sb[0:64, 0:64], in_=pss[0:64, 0:64],
                            func=AF.Exp, scale=inv_sqrt_d,
                            accum_out=den_all[0:64, pr:pr + 1],
                        )
                        nc.scalar.activation(
                            out=p_sb[64:128, 64:128], in_=pss[64:128, 64:128],
                            func=AF.Exp, scale=inv_sqrt_d,
                            accum_out=den_all[64:128, pr:pr + 1],
                        )
                        nc.vector.reciprocal(
                            rec_all[:, pr:pr + 1], den_all[:, pr:pr + 1]
                        )
                        pss_tiles[pr] = pss

                    def p_transpose(pr):
                        ps3 = ps_pool.tile([128, 128], F16, tag="ps")
                        nc.tensor.transpose(ps3, p_pers[pr], ident_sb)
                        pT_sb = soft_pool.tile([128, 128], F16, tag="pT")
                        nc.scalar.copy(out=pT_sb, in_=ps3)
                        pT_tiles[pr] = pT_sb

                    def attn_stats(pr):
                        psa = ps_pool.tile([128, D], F32, tag="ps")
                        nc.tensor.matmul(psa, pT_tiles[pr], ctn[:, pr, :])
                        a_sb = acc_pool.tile([128, D], F16, tag="a")
                        nc.vector.scalar_tensor_tensor(
                            out=a_sb, in0=psa, scalar=rec_all[:, pr:pr + 1],
                            in1=ctn[:, pr, :], op0=ALU.mult, op1=ALU.add,
                        )
                        st = stat_pool.tile([128, 6], F32, tag="st")
                        nc.vector.bn_stats(st, a_sb)
                        nc.vector.bn_aggr(mv_all[:, pr, :], st)
                        a_tiles[pr] = a_sb
                        if debug and row == 0 and s == 0:
                            nc.sync.dma_start(out=dbg["p"][pr, :, :], in_=p_pers[pr])
                            nc.sync.dma_start(out=dbg["pT"][pr, :, :], in_=pT_tiles[pr])

                    PIPE = True
                    if PIPE:
                        for step in range(6):
                            if step < 4:
                                scores_exp(step)
                            if 1 <= step <= 4:
                                p_transpose(step - 1)
                            if step >= 2:
                                attn_stats(step - 2)
                    else:
                        for pr in range(4):
                            scores_exp(pr)
                            p_transpose(pr)
                            attn_stats(pr)

                    def make_tail(mv_all, lnv_all, rstd_all, a_tiles, chunk0):
                        def tail():
                            nc.scalar.activation(
                                out=lnv_all, in_=mv_all[:, :, 1], func=AF.Sqrt,
                                bias=eps_sb, scale=1.0,
                            )
                            nc.vector.reciprocal(rstd_all, lnv_all)
                            psp = psp_pool.tile([8, D], F32, tag="psp")
                            if debug and chunk0 == 0:
                                nc.sync.dma_start(out=dbg["rstd"][:, :], in_=rstd_all)
                            for pr in range(4):
                                a_sb = a_tiles[pr]
                                nc.vector.tensor_scalar(
                                    out=a_sb, in0=a_sb, scalar1=mv_all[:, pr, 0:1],
                                    scalar2=rstd_all[:, pr:pr + 1],
                                    op0=ALU.subtract, op1=ALU.mult,
                                )
                                nc.tensor.matmul(
                                    psp, pw_sb[:, pr * 8:(pr + 1) * 8], a_sb,
                                    start=(pr == 0), stop=(pr == 3),
                                )
                                if debug and chunk0 == 0:
                                    nc.sync.dma_start(
                                        out=dbg["a"][pr, :, :], in_=a_sb.bitcast(F32)
                                    )
                            out_sb = po_pool.tile([8, D], F32, tag="po")
                            nc.vector.tensor_copy(out=out_sb, in_=psp)
                            nc.vector.tensor_mul(out_sb, out_sb, g8)
                            nc.vector.tensor_add(out_sb, out_sb, b8)
                            nc.sync.dma_start(
                                out=out[chunk0:chunk0 + CH, :], in_=out_sb
                            )
                        return tail

                    if debug and row == 0 and s == 0:
                        nc.sync.dma_start(out=dbg["den"][:, :], in_=den_all)
                        nc.sync.dma_start(out=dbg["mv"][:, :, :], in_=mv_all)
                    chunk0 = row * (T // L) + s * CH
                    pending_tail = make_tail(
                        mv_all, lnv_all, rstd_all, list(a_tiles), chunk0
                    )
            if pending_tail is not None:
                pending_tail()
                pending_tail = None

    nc.finalize()
    return nc


def host_inputs(x, conv_w, conv_b, ln_g, ln_b, n_cores):
    """Shard + transform inputs for the device program."""
    B = x.shape[0]
    b_loc = B // n_cores
    wt = np.transpose(np.asarray(conv_w, dtype=np.float32), (2, 1, 0)).reshape(
        2 * D, D
    ).astype(np.float16)
    # device loads wt as [p, k, o] with row index p*8+k: row p*8+k must hold
    # original wt row k*128+p (k = tap*4 + i_block)
    idx = (np.arange(1024) % 8) * 128 + (np.arange(1024) // 8)
    wt = np.ascontiguousarray(wt[idx])
    pw = np.zeros((128, 32), dtype=np.float16)
    for tp in range(128):
        for p in range(4):
            pw[tp, p * 8 + 2 * p + tp // 64] = 1.0 / 64
    ident = np.eye(128, dtype=np.float16)
    x16 = np.asarray(x, dtype=np.float16)
    common = {
        "wt": wt,
        "cb": np.ascontiguousarray(np.asarray(conv_b, dtype=np.float32)),
        "g": np.ascontiguousarray(np.asarray(ln_g, dtype=np.float32)),
        "bb": np.ascontiguousarray(np.asarray(ln_b, dtype=np.float32)),
        "pw": pw,
        "ident": ident,
    }
    in_maps = []
    for c in range(n_cores):
        m = dict(common)
        m["x"] = np.ascontiguousarray(x16[c * b_loc:(c + 1) * b_loc])
        in_maps.append(m)
    return in_maps


def kernel(x, conv_w, conv_b, ln_g, ln_b, n_stages):
    x = np.asarray(x)
    B, T, d = x.shape
    assert d == D and int(n_stages) * L == T, (x.shape, n_stages)
    n_cores = 8
    nc = build_program(B // n_cores, T, reps=1)
    in_maps = host_inputs(x, conv_w, conv_b, ln_g, ln_b, n_cores)
    res = run_bass_kernel_spmd(nc, in_maps, list(range(n_cores)))
    outs = [res.results[c]["out"] for c in range(n_cores)]
    full = np.concatenate(outs, axis=0)  # [B*n_stages, D]
    return full.reshape(B, int(n_stages), D).astype(np.float32)


if __name__ == "__main__":
    rng = np.random.default_rng(0)
    x = rng.standard_normal((32, 4096, D), dtype=np.float32)
    conv_w = (rng.standard_normal((D, D, 2)) / np.sqrt(2 * D)).astype(np.float32)
    conv_b = (rng.standard_normal(D) * 0.02).astype(np.float32)
    out = kernel(x, conv_w, conv_b, np.ones(D, np.float32), np.zeros(D, np.float32), 64)
    print(out.shape, out.dtype)
